# revision 6
# baseline (speedup 1.0000x reference)
"""Trainium2 Bass kernel for nn_AttentionPooling (segment-softmax attention pooling).

Math restructuring (vs the reference):
  scores[n,h] = (x @ Wk.T + bk) . pool_query  * scale  ==  x @ As + c0
      with As[j,h] = scale * sum_d Wk[h*HD+d, j] * pq[h,d]   (tiny [256,8])
  e = exp(scores)            (no max-subtraction needed; |scores| < ~6)
  z[c,h]   = sum_{n in c} e[n,h]
  U[c,h,:] = sum_{n in c} e[n,h] * x[n,:]
  T = U / z;  ssum[c, h*HD:(h+1)*HD] = T[c,h,:] @ Wv_h.T + bv_h   (softmax weights
      sum to 1 per segment, so the bias term is exact)
  pooled = ssum / max(cnt,1);  table = pooled @ Wo.T + bo;  out = table[seg]

Division of labor:
  Host (cheap O(N*8) / O(C) work): the tiny rank-8 score projection (one BLAS
      sgemm) + exp, cluster bin-packing, building the bf16 upload buffer, the
      [C, HID] table math, and the final out = table[seg] expansion.
  Device (the memory-bound segment reduction over all of x, SPMD on 8 cores):
      per 128-node tile, DVE builds the "weighted indicator"
      wind[n, slot*8+h] = (segw[n]==slot) * e[n,h] from the DMA'd e columns,
      and PE accumulates U/z with one bf16 matmul per tile:
      uz[slot*8+h, :] += wind.T @ [x | 1].  Clusters are host-bin-packed into
      8-per-core windows of <=16 clusters, each padded to exactly T_W tiles.

Everything the device touches is bf16 (x, e, masks): halves HBM traffic (the
bottleneck) and doubles DVE throughput; accumulation stays fp32 in PSUM.
The upload buffer is row-swizzled so each SBUF partition's chunk DMA is one
contiguous 4256B read from HBM.
"""

import sys

if "/opt/trn_rl_repo" not in sys.path:
    sys.path.insert(0, "/opt/trn_rl_repo")

import numpy as np
from contextlib import ExitStack

import concourse.bass as bass
import concourse.tile as tile
from concourse import bacc, mybir

F32 = mybir.dt.float32
BF16 = mybir.dt.bfloat16

# Problem constants
N = 200000
HID = 256
HEADS = 8
HD = HID // HEADS
C = 1000
SCALE = HD ** -0.5

# Sharding constants
N_CORES = 8
TILE = 128
SLOTS = 16          # clusters per window
N_WIN = 8           # windows per core  (8*16*8 = 1024 cluster slots >= 1000)
T_W = 26            # tiles per window (padded)
NTILES = N_WIN * T_W           # 208 tiles per core
NL = NTILES * TILE             # 26624 nodes per core (padded)
PAD_SLOT = 255.0

COLS = HID + 10     # per-row upload: 256 x | 1 ones | 8 e | 1 pad  (532 B)
CH = 8              # tiles per DMA chunk


# ----------------------------------------------------------------------------
# Device program
# ----------------------------------------------------------------------------

def build_main_program(n_win=N_WIN, t_w=T_W, repeat=1, hw_loop=0, stage="full",
                       ch=CH, x_bufs=4, wind_bufs=6, mask_engine="vector",
                       pair=True):
    """Single-sweep program over ntiles 128-node tiles: per tile, build the
    weighted indicator wind on DVE and accumulate U/z with one bf16 matmul.
    `repeat`/`hw_loop` re-run the whole sweep (for timing); `stage` truncates
    the per-tile pipeline ("dma" / "mask" / "full")."""
    ntiles = n_win * t_w
    nc = bacc.Bacc("TRN2", target_bir_lowering=False, debug=False,
                   enable_asserts=False, num_devices=N_CORES)

    # host-swizzled: row p of chunk chb = [tile chb*ch+0..ch-1, node-lane p]
    x_d = nc.dram_tensor("x", [(ntiles // ch) * TILE, ch * COLS], BF16,
                         kind="ExternalInput").ap()
    segw_d = nc.dram_tensor("segw", [TILE, ntiles], BF16, kind="ExternalInput").ap()
    win16_d = nc.dram_tensor("win16", [TILE, 2 * TILE], BF16,
                             kind="ExternalInput").ap()
    U_d = nc.dram_tensor("U", [TILE, n_win * HID], F32, kind="ExternalOutput").ap()
    z_d = nc.dram_tensor("z", [TILE, n_win], F32, kind="ExternalOutput").ap()

    with tile.TileContext(nc) as tc, ExitStack() as ctx:
        consts = ctx.enter_context(tc.tile_pool(name="consts", bufs=1))
        accs = ctx.enter_context(tc.tile_pool(name="accs", bufs=1))
        x_pool = ctx.enter_context(tc.tile_pool(name="xc", bufs=x_bufs))
        mask_pool = ctx.enter_context(tc.tile_pool(name="mask", bufs=wind_bufs))
        wind_pool = ctx.enter_context(tc.tile_pool(name="wind", bufs=wind_bufs))
        uz_psum = ctx.enter_context(tc.tile_pool(name="uzps", bufs=2, space="PSUM"))

        win16_sb = consts.tile([TILE, 2 * TILE], BF16)
        nc.sync.dma_start(win16_sb[:], win16_d[:])
        segw_sb = consts.tile([TILE, ntiles], BF16)
        nc.sync.dma_start(segw_sb[:], segw_d[:])

        U_sb = accs.tile([TILE, n_win * HID], F32)
        z_sb = accs.tile([TILE, n_win], F32)
        if stage != "full":
            nc.vector.memset(U_sb[:], 0.0)
            nc.vector.memset(z_sb[:], 0.0)

        mask_eng = nc.gpsimd if mask_engine == "gpsimd" else nc.vector

        def sweep():
            uz_cur = None
            for chb in range(ntiles // ch):
                xc = x_pool.tile([TILE, ch * COLS], BF16, tag="xc")
                nc.sync.dma_start(xc[:], x_d[chb * TILE:(chb + 1) * TILE, :])
                if stage == "dma":
                    continue
                # masks for a pair of tiles in one DVE op (halved op overhead)
                step = 2 if (pair and ch % 2 == 0) else 1
                for a2 in range(0, ch, step):
                    mask = mask_pool.tile([TILE, step * TILE], BF16, tag="mask")
                    t0 = chb * ch + a2
                    if step == 2:
                        segw_rep = segw_sb[:, t0:t0 + 2].unsqueeze(-1) \
                            .to_broadcast([TILE, 2, TILE])
                        win2 = win16_sb[:].rearrange("p (a j) -> p a j", a=2)
                        mv = mask[:].rearrange("p (a j) -> p a j", a=2)
                        mask_eng.tensor_tensor(mv, win2, segw_rep,
                                               op=mybir.AluOpType.is_equal)
                    else:
                        mask_eng.tensor_scalar(mask[:], win16_sb[:, 0:TILE],
                                               segw_sb[:, t0:t0 + 1], None,
                                               op0=mybir.AluOpType.is_equal)
                    for i in range(step):
                        a = a2 + i
                        t = chb * ch + a
                        w, ti = divmod(t, t_w)
                        base = a * COLS
                        wind = wind_pool.tile([TILE, TILE], BF16, tag="wind")
                        e_rep = xc[:, base + HID + 1:base + HID + 9] \
                            .unsqueeze(1).to_broadcast([TILE, SLOTS, HEADS])
                        nc.vector.tensor_tensor(
                            wind[:], mask[:, i * TILE:(i + 1) * TILE], e_rep,
                            op=mybir.AluOpType.mult)
                        if stage == "mask":
                            continue
                        if ti == 0:
                            uz_cur = uz_psum.tile([TILE, HID + 1], F32, tag="uz")
                        nc.tensor.matmul(uz_cur[:], lhsT=wind[:],
                                         rhs=xc[:, base:base + HID + 1],
                                         start=(ti == 0), stop=(ti == t_w - 1))
                        if ti == t_w - 1:
                            nc.scalar.copy(U_sb[:, w * HID:(w + 1) * HID],
                                           uz_cur[:, 0:HID])
                            nc.scalar.copy(z_sb[:, w:w + 1],
                                           uz_cur[:, HID:HID + 1])

        if hw_loop > 1:
            with tc.For_i(0, hw_loop, 1):
                sweep()
        else:
            for _rep in range(repeat):
                sweep()

        nc.sync.dma_start(U_d[:], U_sb[:])
        nc.sync.dma_start(z_d[:], z_sb[:])

    nc.compile()
    return nc


# Rebind the builder from source exec'd under a fixed pseudo-filename: the
# BIR embeds op debug info (filename/lineno), so building from the real file
# path would make the compiled program differ per directory and miss the
# neuronxcc compile cache.
import inspect as _inspect

_builder_src = _inspect.getsource(build_main_program)
exec(compile(_builder_src, "<akp_builders>", "exec"), globals())


# ----------------------------------------------------------------------------
# Host-side planning
# ----------------------------------------------------------------------------

def plan_sharding(ca):
    """Bin-pack 1000 clusters into 64 (core, window) bins, <=16 clusters and
    <= T_W*128 nodes per bin. Returns per-core node index arrays + slot maps."""
    counts = np.bincount(ca, minlength=C)
    order = np.argsort(counts, kind="stable")[::-1]
    nbins = N_CORES * N_WIN
    cap = T_W * TILE
    loads = np.zeros(nbins, dtype=np.int64)
    nslots = np.zeros(nbins, dtype=np.int64)
    bin_clusters = [[] for _ in range(nbins)]
    for c in order:
        # least-loaded bin with a free slot
        cand = np.where(nslots < SLOTS)[0]
        b = cand[np.argmin(loads[cand])]
        bin_clusters[b].append(int(c))
        loads[b] += counts[c]
        nslots[b] += 1
    assert loads.max() <= cap, f"bin overflow: {loads.max()} > {cap}"

    # node lists per cluster (sorted order)
    idx_sorted = np.argsort(ca, kind="stable")
    starts = np.zeros(C + 1, dtype=np.int64)
    np.cumsum(counts, out=starts[1:])

    node_idx = np.full((N_CORES, NL), -1, dtype=np.int64)
    segw = np.full((N_CORES, NL), PAD_SLOT, dtype=np.float32)
    slot_cluster = np.full((N_CORES, N_WIN, SLOTS), -1, dtype=np.int64)
    for b in range(nbins):
        core, w = divmod(b, N_WIN)
        pos = w * cap
        for s, c in enumerate(bin_clusters[b]):
            slot_cluster[core, w, s] = c
            m = counts[c]
            node_idx[core, pos:pos + m] = idx_sorted[starts[c]:starts[c] + m]
            segw[core, pos:pos + m] = s
            pos += m
    return node_idx, segw, slot_cluster, counts


def host_table_math(U_all, z_all, slot_cluster, counts, Wv, bv, Wo, bo):
    """[8,128,2048] U + [8,128,8] z -> projected per-cluster table [C, HID]."""
    # U[core][row=s*8+h, col=w*256+j] ; z[core][row, w]
    U5 = U_all.reshape(N_CORES, SLOTS, HEADS, N_WIN, HID)
    z4 = z_all.reshape(N_CORES, SLOTS, HEADS, N_WIN)
    Uc = np.zeros((C, HEADS, HID), dtype=np.float64)
    zc = np.zeros((C, HEADS), dtype=np.float64)
    sc = slot_cluster  # [core, w, s]
    valid = sc >= 0
    cores, ws, ss = np.nonzero(valid)
    cl = sc[cores, ws, ss]
    Uc[cl] = U5[cores, ss, :, ws, :]
    zc[cl] = z4[cores, ss, :, ws]
    zc_safe = np.where(zc > 0, zc, 1.0)
    T = Uc / zc_safe[:, :, None]                      # [C, H, HID]
    Wv_r = np.asarray(Wv, np.float64).reshape(HEADS, HD, HID)
    ssum = np.einsum("chj,hdj->chd", T, Wv_r)         # [C, H, HD]
    ssum += np.asarray(bv, np.float64).reshape(HEADS, HD)[None]
    ssum = ssum.reshape(C, HID)
    ssum[counts == 0] = 0.0
    pooled = ssum / np.maximum(counts, 1)[:, None]
    table = pooled @ np.asarray(Wo, np.float64).T + np.asarray(bo, np.float64)
    return table.astype(np.float32)


_CACHE = {}


def make_runner(nc, n_cores=N_CORES):
    """Persistent jitted runner for a compiled Bacc program (axon/PJRT path).

    Same mechanism as run_bass_kernel_spmd's axon redirect (bass2jax), but the
    jitted executable is built once and reused, so steady-state calls skip
    retracing/lowering."""
    import jax
    from jax.sharding import Mesh, PartitionSpec, NamedSharding
    from jax.experimental.shard_map import shard_map
    from concourse.bass2jax import (_bass_exec_p, install_neuronx_cc_hook,
                                    partition_id_tensor)

    install_neuronx_cc_hook()
    in_names, out_names, out_avals = [], [], []
    partition_name = nc.partition_id_tensor.name if nc.partition_id_tensor else None
    for alloc in nc.m.functions[0].allocations:
        if not isinstance(alloc, mybir.MemoryLocationSet):
            continue
        name = alloc.memorylocations[0].name
        if alloc.kind == "ExternalInput":
            if name != partition_name:
                in_names.append(name)
        elif alloc.kind == "ExternalOutput":
            out_names.append(name)
            shape = tuple(alloc.tensor_shape)
            dtype = mybir.dt.np(alloc.dtype)
            out_avals.append(jax.core.ShapedArray(shape, dtype))
    n_params = len(in_names)
    n_outs = len(out_avals)
    all_in_names = list(in_names) + list(out_names)
    if partition_name:
        all_in_names.append(partition_name)

    def _body(*args):
        operands = list(args)
        if partition_name:
            operands.append(partition_id_tensor())
        return tuple(_bass_exec_p.bind(
            *operands, out_avals=tuple(out_avals), in_names=tuple(all_in_names),
            out_names=tuple(out_names), lowering_input_output_aliases=(),
            sim_require_finite=True, sim_require_nnan=True, nc=nc))

    devices = jax.devices()[:n_cores]
    mesh = Mesh(np.asarray(devices), ("core",))
    donate = tuple(range(n_params, n_params + n_outs))
    sharded = jax.jit(
        shard_map(_body, mesh=mesh,
                  in_specs=(PartitionSpec("core"),) * (n_params + n_outs),
                  out_specs=(PartitionSpec("core"),) * n_outs, check_rep=False),
        donate_argnums=donate, keep_unused=True)
    sharding = NamedSharding(mesh, PartitionSpec("core"))
    zero_shapes = [(n_cores * a.shape[0], *a.shape[1:]) for a in out_avals]
    zero_dtypes = [a.dtype for a in out_avals]

    def run(in_maps, pre=None):
        """in_maps: per-core dicts of np arrays. pre: dict name -> global jax
        Array (already sharded) taking precedence over in_maps."""
        import jax as _jax
        pre = pre or {}
        concat_in = []
        for name in in_names:
            if name in pre:
                concat_in.append(pre[name])
            else:
                concat_in.append(np.concatenate(
                    [np.asarray(m[name]) for m in in_maps], axis=0))
        zs = [_jax.device_put(np.zeros(s, d), sharding)
              for s, d in zip(zero_shapes, zero_dtypes)]
        outs = _jax.block_until_ready(sharded(*concat_in, *zs))
        return [{name: np.asarray(outs[i]).reshape(n_cores, *out_avals[i].shape)[c]
                 for i, name in enumerate(out_names)}
                for c in range(n_cores)]

    run.devices = devices
    run.sharding = sharding
    return run


def _get_programs():
    if "main" not in _CACHE:
        _CACHE["main"] = build_main_program()
        _CACHE["main_run"] = make_runner(_CACHE["main"])
    return _CACHE


# ----------------------------------------------------------------------------
# Entry point
# ----------------------------------------------------------------------------

def kernel(x, cluster_assignments, batch, Wk, bk, Wv, bv, Wo, bo, pool_query):
    import ml_dtypes
    bf16 = ml_dtypes.bfloat16

    x = np.ascontiguousarray(np.asarray(x, dtype=np.float32))
    ca = np.asarray(cluster_assignments).astype(np.int64)
    Wk = np.asarray(Wk, np.float32)
    bk = np.asarray(bk, np.float32)
    pq = np.asarray(pool_query, np.float32)[0]  # [H, HD]

    # folded score projection (tiny): scores = x @ As + c0
    As = (np.asarray(Wk, np.float64).reshape(HEADS, HD, HID)
          * np.asarray(pq, np.float64)[:, :, None]).sum(1)     # [H, HID]
    As = (As.T * SCALE).astype(np.float32)                     # [HID, H]
    c0 = ((np.asarray(bk, np.float64).reshape(HEADS, HD)
           * np.asarray(pq, np.float64)).sum(1) * SCALE).astype(np.float32)
    e = np.exp(x @ As + c0)                                    # [N, 8] f32

    progs = _get_programs()
    run = progs["main_run"]

    node_idx, segw, slot_cluster, counts = plan_sharding(ca)

    xpad = np.vstack([x, np.zeros((1, HID), np.float32)])
    epad = np.vstack([e, np.zeros((1, HEADS), np.float32)])
    nip = np.where(node_idx >= 0, node_idx, N).reshape(-1)     # [8*NL]

    # upload rows: [x(256) | 1 | e(8) | 0pad] in bf16
    x_big = np.empty((N_CORES * NL, COLS), bf16)
    x_big[:, 0:HID] = xpad[nip]
    x_big[:, HID] = 1.0
    x_big[:, HID + 1:HID + 9] = epad[nip]
    x_big[:, HID + 9] = 0.0
    # swizzle: rows of each ch-tile chunk so partition p's DMA is contiguous
    x_big = np.ascontiguousarray(
        x_big.reshape(N_CORES, NTILES // CH, CH, TILE, COLS)
        .transpose(0, 1, 3, 2, 4)
        .reshape(N_CORES * (NTILES // CH) * TILE, CH * COLS))

    win16 = np.repeat(np.arange(SLOTS, dtype=np.float32), HEADS)[None, :] \
        .repeat(TILE, 0).astype(bf16)                           # [128, 128]
    win16 = np.ascontiguousarray(np.tile(win16, (1, 2)))        # [128, 256]
    in_maps = []
    for core in range(N_CORES):
        segw_core = np.ascontiguousarray(
            segw[core].reshape(NTILES, TILE).T).astype(bf16)    # [128, NTILES]
        in_maps.append({"segw": segw_core, "win16": win16})

    results = run(in_maps, pre={"x": x_big})
    U_all = np.stack([results[i]["U"] for i in range(N_CORES)])
    z_all = np.stack([results[i]["z"] for i in range(N_CORES)])

    table = host_table_math(U_all, z_all, slot_cluster, counts, Wv, bv, Wo, bo)

    out = np.empty((N, HID), dtype=np.float32)
    np.take(table, ca, axis=0, out=out)
    return out


# revision 14
# speedup vs baseline: 9.2532x; 9.2532x over previous
"""Trainium2 Bass kernel for nn_AttentionPooling (segment-softmax attention pooling).

Math restructuring (vs the reference):
  scores[n,h] = (x @ Wk.T + bk) . pool_query  * scale  ==  x @ As + c0
      with As[j,h] = scale * sum_d Wk[h*HD+d, j] * pq[h,d]   (tiny [256,8])
  e = exp(scores)            (no max-subtraction needed; |scores| < ~6)
  z[c,h]   = sum_{n in c} e[n,h]
  U[c,h,:] = sum_{n in c} e[n,h] * x[n,:]
  T = U / z;  ssum[c, h*HD:(h+1)*HD] = T[c,h,:] @ Wv_h.T + bv_h   (softmax weights
      sum to 1 per segment, so the bias term is exact)
  pooled = ssum / max(cnt,1);  table = pooled @ Wo.T + bo;  out = table[seg]

Division of labor:
  Host (cheap O(N*8) / O(C) work): the tiny rank-8 score projection (one BLAS
      sgemm) + exp, cluster bin-packing, building the bf16 upload buffer, the
      [C, HID] table math, and the final out = table[seg] expansion.
  Device (the memory-bound segment reduction over all of x, SPMD on 8 cores):
      per 128-node tile, DVE builds the "weighted indicator"
      wind[n, slot*8+h] = (segw[n]==slot) * e[n,h] from the DMA'd e columns,
      and PE accumulates U/z with one bf16 matmul per tile:
      uz[slot*8+h, :] += wind.T @ [x | 1].  Clusters are host-bin-packed into
      8-per-core windows of <=16 clusters, each padded to exactly T_W tiles.

Everything the device touches is bf16 (x, e, masks): halves HBM traffic (the
bottleneck) and doubles DVE throughput; accumulation stays fp32 in PSUM.
The upload buffer is row-swizzled so each SBUF partition's chunk DMA is one
contiguous 4256B read from HBM.
"""

import sys

if "/opt/trn_rl_repo" not in sys.path:
    sys.path.insert(0, "/opt/trn_rl_repo")

import numpy as np
from contextlib import ExitStack

import concourse.bass as bass
import concourse.tile as tile
from concourse import bacc, mybir

F32 = mybir.dt.float32
BF16 = mybir.dt.bfloat16

# Problem constants
N = 200000
HID = 256
HEADS = 8
HD = HID // HEADS
C = 1000
SCALE = HD ** -0.5

# Sharding constants
N_CORES = 8
TILE = 128
SLOTS = 16          # clusters per window
N_WIN = 8           # windows per core  (8*16*8 = 1024 cluster slots >= 1000)
T_W = 26            # tiles per window (padded)
NTILES = N_WIN * T_W           # 208 tiles per core
NL = NTILES * TILE             # 26624 nodes per core (padded)
PAD_SLOT = 255.0

COLS = HID + 10     # per-row upload: 256 x | 1 ones | 8 e | 1 pad  (532 B)
CH = 8              # tiles per DMA chunk


# ----------------------------------------------------------------------------
# Device program
# ----------------------------------------------------------------------------

def build_main_program(n_win=N_WIN, t_w=T_W, repeat=1, hw_loop=0, stage="full",
                       ch=CH, x_bufs=4, wind_bufs=6, mask_engine="vector",
                       pair=True, fuse=False, batch=True):
    """Single-sweep program over ntiles 128-node tiles: per tile, build the
    weighted indicator wind on DVE and accumulate U/z with one bf16 matmul.
    `repeat`/`hw_loop` re-run the whole sweep (for timing); `stage` truncates
    the per-tile pipeline ("dma" / "mask" / "full")."""
    ntiles = n_win * t_w
    nc = bacc.Bacc("TRN2", target_bir_lowering=False, debug=False,
                   enable_asserts=False, num_devices=N_CORES)

    # host-swizzled: row p of chunk chb = [tile chb*ch+0..ch-1, node-lane p]
    x_d = nc.dram_tensor("x", [(ntiles // ch) * TILE, ch * COLS], BF16,
                         kind="ExternalInput").ap()
    segw_d = nc.dram_tensor("segw", [TILE, ntiles], BF16, kind="ExternalInput").ap()
    win16_d = nc.dram_tensor("win16", [TILE, max(2, ch) * TILE], BF16,
                             kind="ExternalInput").ap()
    U_d = nc.dram_tensor("U", [TILE, n_win * HID], F32, kind="ExternalOutput").ap()
    z_d = nc.dram_tensor("z", [TILE, n_win], F32, kind="ExternalOutput").ap()

    with tile.TileContext(nc) as tc, ExitStack() as ctx:
        consts = ctx.enter_context(tc.tile_pool(name="consts", bufs=1))
        accs = ctx.enter_context(tc.tile_pool(name="accs", bufs=1))
        x_pool = ctx.enter_context(tc.tile_pool(name="xc", bufs=x_bufs))
        mask_pool = ctx.enter_context(tc.tile_pool(name="mask", bufs=wind_bufs))
        wind_pool = ctx.enter_context(tc.tile_pool(name="wind", bufs=wind_bufs))
        sg_pool = ctx.enter_context(tc.tile_pool(name="sg", bufs=3))
        uz_psum = ctx.enter_context(tc.tile_pool(name="uzps", bufs=2, space="PSUM"))

        win16_sb = consts.tile([TILE, max(2, ch) * TILE], BF16)
        nc.sync.dma_start(win16_sb[:], win16_d[:])
        segw_sb = consts.tile([TILE, ntiles], BF16)
        nc.sync.dma_start(segw_sb[:], segw_d[:])

        U_sb = accs.tile([TILE, n_win * HID], F32)
        z_sb = accs.tile([TILE, n_win], F32)
        if stage != "full":
            nc.vector.memset(U_sb[:], 0.0)
            nc.vector.memset(z_sb[:], 0.0)

        mask_eng = nc.gpsimd if mask_engine == "gpsimd" else nc.vector

        def sweep():
            uz_cur = None
            for chb in range(ntiles // ch):
                xc = x_pool.tile([TILE, ch * COLS], BF16, tag="xc")
                nc.sync.dma_start(xc[:], x_d[chb * TILE:(chb + 1) * TILE, :])
                if stage == "dma":
                    continue
                if batch:
                    # batched DVE: 3 ops per chunk, all TT-2x-eligible
                    # (every operand's innermost dim is the contiguous h)
                    t0 = chb * ch
                    sg8 = sg_pool.tile([TILE, ch * HEADS], BF16, tag="sg")
                    sgv = sg8[:].rearrange("p (a h) -> p a h", a=ch)
                    nc.vector.tensor_copy(
                        sgv, segw_sb[:, t0:t0 + ch].unsqueeze(-1)
                        .to_broadcast([TILE, ch, HEADS]))
                    mask8 = mask_pool.tile([TILE, ch * TILE], BF16, tag="mask")
                    m4 = mask8[:].rearrange("p (a s h) -> p a s h",
                                            a=ch, s=SLOTS)
                    win_v = win16_sb[:, 0:ch * TILE].rearrange(
                        "p (a s h) -> p a s h", a=ch, s=SLOTS)
                    sg_b = sgv.unsqueeze(2).to_broadcast(
                        [TILE, ch, SLOTS, HEADS])
                    nc.vector.tensor_tensor(m4, win_v, sg_b,
                                            op=mybir.AluOpType.is_equal)
                    wind8 = wind_pool.tile([TILE, ch * TILE], BF16, tag="wind")
                    w4 = wind8[:].rearrange("p (a s h) -> p a s h",
                                            a=ch, s=SLOTS)
                    ev = xc[:].rearrange("p (a j) -> p a j", a=ch) \
                        [:, :, HID + 1:HID + 9].unsqueeze(2) \
                        .to_broadcast([TILE, ch, SLOTS, HEADS])
                    nc.vector.tensor_tensor(w4, m4, ev,
                                            op=mybir.AluOpType.mult)
                    if stage == "mask":
                        continue
                    for a in range(ch):
                        t = chb * ch + a
                        w, ti = divmod(t, t_w)
                        base = a * COLS
                        if ti == 0:
                            uz_cur = uz_psum.tile([TILE, HID + 1], F32, tag="uz")
                        nc.tensor.matmul(uz_cur[:],
                                         lhsT=wind8[:, a * TILE:(a + 1) * TILE],
                                         rhs=xc[:, base:base + HID + 1],
                                         start=(ti == 0), stop=(ti == t_w - 1))
                        if ti == t_w - 1:
                            nc.scalar.copy(U_sb[:, w * HID:(w + 1) * HID],
                                           uz_cur[:, 0:HID])
                            nc.scalar.copy(z_sb[:, w:w + 1],
                                           uz_cur[:, HID:HID + 1])
                    continue
                if fuse:
                    # one fused DVE op per tile: wind = (win16==segw[t]) * e
                    for a in range(ch):
                        t = chb * ch + a
                        w, ti = divmod(t, t_w)
                        base = a * COLS
                        wind = wind_pool.tile([TILE, TILE], BF16, tag="wind")
                        e_rep = xc[:, base + HID + 1:base + HID + 9] \
                            .unsqueeze(1).to_broadcast([TILE, SLOTS, HEADS])
                        wv = wind[:].rearrange("p (s h) -> p s h", s=SLOTS)
                        win1 = win16_sb[:, 0:TILE] \
                            .rearrange("p (s h) -> p s h", s=SLOTS)
                        nc.vector.scalar_tensor_tensor(
                            wv, win1, segw_sb[:, t:t + 1], e_rep,
                            op0=mybir.AluOpType.is_equal,
                            op1=mybir.AluOpType.mult)
                        if stage == "mask":
                            continue
                        if ti == 0:
                            uz_cur = uz_psum.tile([TILE, HID + 1], F32, tag="uz")
                        nc.tensor.matmul(uz_cur[:], lhsT=wind[:],
                                         rhs=xc[:, base:base + HID + 1],
                                         start=(ti == 0), stop=(ti == t_w - 1))
                        if ti == t_w - 1:
                            nc.scalar.copy(U_sb[:, w * HID:(w + 1) * HID],
                                           uz_cur[:, 0:HID])
                            nc.scalar.copy(z_sb[:, w:w + 1],
                                           uz_cur[:, HID:HID + 1])
                    continue
                # unfused: masks for a pair of tiles in one DVE op
                step = 2 if (pair and ch % 2 == 0) else 1
                for a2 in range(0, ch, step):
                    mask = mask_pool.tile([TILE, step * TILE], BF16, tag="mask")
                    t0 = chb * ch + a2
                    if step == 2:
                        segw_rep = segw_sb[:, t0:t0 + 2].unsqueeze(-1) \
                            .to_broadcast([TILE, 2, TILE])
                        win2 = win16_sb[:].rearrange("p (a j) -> p a j", a=2)
                        mv = mask[:].rearrange("p (a j) -> p a j", a=2)
                        mask_eng.tensor_tensor(mv, win2, segw_rep,
                                               op=mybir.AluOpType.is_equal)
                    else:
                        mask_eng.tensor_scalar(mask[:], win16_sb[:, 0:TILE],
                                               segw_sb[:, t0:t0 + 1], None,
                                               op0=mybir.AluOpType.is_equal)
                    for i in range(step):
                        a = a2 + i
                        t = chb * ch + a
                        w, ti = divmod(t, t_w)
                        base = a * COLS
                        wind = wind_pool.tile([TILE, TILE], BF16, tag="wind")
                        e_rep = xc[:, base + HID + 1:base + HID + 9] \
                            .unsqueeze(1).to_broadcast([TILE, SLOTS, HEADS])
                        nc.vector.tensor_tensor(
                            wind[:], mask[:, i * TILE:(i + 1) * TILE], e_rep,
                            op=mybir.AluOpType.mult)
                        if stage == "mask":
                            continue
                        if ti == 0:
                            uz_cur = uz_psum.tile([TILE, HID + 1], F32, tag="uz")
                        nc.tensor.matmul(uz_cur[:], lhsT=wind[:],
                                         rhs=xc[:, base:base + HID + 1],
                                         start=(ti == 0), stop=(ti == t_w - 1))
                        if ti == t_w - 1:
                            nc.scalar.copy(U_sb[:, w * HID:(w + 1) * HID],
                                           uz_cur[:, 0:HID])
                            nc.scalar.copy(z_sb[:, w:w + 1],
                                           uz_cur[:, HID:HID + 1])

        if hw_loop > 1:
            with tc.For_i(0, hw_loop, 1):
                sweep()
        else:
            for _rep in range(repeat):
                sweep()

        nc.sync.dma_start(U_d[:], U_sb[:])
        nc.sync.dma_start(z_d[:], z_sb[:])

    nc.compile()
    return nc


# Rebind the builder from source exec'd under a fixed pseudo-filename: the
# BIR embeds op debug info (filename/lineno), so building from the real file
# path would make the compiled program differ per directory and miss the
# neuronxcc compile cache.
import inspect as _inspect

_builder_src = _inspect.getsource(build_main_program)
exec(compile(_builder_src, "<akp_builders>", "exec"), globals())


# ----------------------------------------------------------------------------
# Host-side planning
# ----------------------------------------------------------------------------

def plan_sharding(ca):
    """Bin-pack 1000 clusters into 64 (core, window) bins, <=16 clusters and
    <= T_W*128 nodes per bin. Returns per-core node index arrays + slot maps."""
    counts = np.bincount(ca, minlength=C)
    order = np.argsort(counts, kind="stable")[::-1]
    nbins = N_CORES * N_WIN
    cap = T_W * TILE
    loads = np.zeros(nbins, dtype=np.int64)
    nslots = np.zeros(nbins, dtype=np.int64)
    bin_clusters = [[] for _ in range(nbins)]
    for c in order:
        # least-loaded bin with a free slot
        cand = np.where(nslots < SLOTS)[0]
        b = cand[np.argmin(loads[cand])]
        bin_clusters[b].append(int(c))
        loads[b] += counts[c]
        nslots[b] += 1
    assert loads.max() <= cap, f"bin overflow: {loads.max()} > {cap}"

    # node lists per cluster (sorted order)
    idx_sorted = np.argsort(ca, kind="stable")
    starts = np.zeros(C + 1, dtype=np.int64)
    np.cumsum(counts, out=starts[1:])

    node_idx = np.full((N_CORES, NL), -1, dtype=np.int64)
    segw = np.full((N_CORES, NL), PAD_SLOT, dtype=np.float32)
    slot_cluster = np.full((N_CORES, N_WIN, SLOTS), -1, dtype=np.int64)
    for b in range(nbins):
        core, w = divmod(b, N_WIN)
        pos = w * cap
        for s, c in enumerate(bin_clusters[b]):
            slot_cluster[core, w, s] = c
            m = counts[c]
            node_idx[core, pos:pos + m] = idx_sorted[starts[c]:starts[c] + m]
            segw[core, pos:pos + m] = s
            pos += m
    return node_idx, segw, slot_cluster, counts


def host_table_math(U_all, z_all, slot_cluster, counts, Wv, bv, Wo, bo):
    """[8,128,2048] U + [8,128,8] z -> projected per-cluster table [C, HID]."""
    # U[core][row=s*8+h, col=w*256+j] ; z[core][row, w]
    U5 = U_all.reshape(N_CORES, SLOTS, HEADS, N_WIN, HID)
    z4 = z_all.reshape(N_CORES, SLOTS, HEADS, N_WIN)
    Uc = np.zeros((C, HEADS, HID), dtype=np.float64)
    zc = np.zeros((C, HEADS), dtype=np.float64)
    sc = slot_cluster  # [core, w, s]
    valid = sc >= 0
    cores, ws, ss = np.nonzero(valid)
    cl = sc[cores, ws, ss]
    Uc[cl] = U5[cores, ss, :, ws, :]
    zc[cl] = z4[cores, ss, :, ws]
    zc_safe = np.where(zc > 0, zc, 1.0)
    T = Uc / zc_safe[:, :, None]                      # [C, H, HID]
    Wv_r = np.asarray(Wv, np.float64).reshape(HEADS, HD, HID)
    ssum = np.einsum("chj,hdj->chd", T, Wv_r)         # [C, H, HD]
    ssum += np.asarray(bv, np.float64).reshape(HEADS, HD)[None]
    ssum = ssum.reshape(C, HID)
    ssum[counts == 0] = 0.0
    pooled = ssum / np.maximum(counts, 1)[:, None]
    table = pooled @ np.asarray(Wo, np.float64).T + np.asarray(bo, np.float64)
    return table.astype(np.float32)


_CACHE = {}


def make_runner(nc, n_cores=N_CORES):
    """Persistent jitted runner for a compiled Bacc program (axon/PJRT path).

    Same mechanism as run_bass_kernel_spmd's axon redirect (bass2jax), but the
    jitted executable is built once and reused, so steady-state calls skip
    retracing/lowering."""
    import jax
    from jax.sharding import Mesh, PartitionSpec, NamedSharding
    from jax.experimental.shard_map import shard_map
    from concourse.bass2jax import (_bass_exec_p, install_neuronx_cc_hook,
                                    partition_id_tensor)

    install_neuronx_cc_hook()
    in_names, out_names, out_avals = [], [], []
    partition_name = nc.partition_id_tensor.name if nc.partition_id_tensor else None
    for alloc in nc.m.functions[0].allocations:
        if not isinstance(alloc, mybir.MemoryLocationSet):
            continue
        name = alloc.memorylocations[0].name
        if alloc.kind == "ExternalInput":
            if name != partition_name:
                in_names.append(name)
        elif alloc.kind == "ExternalOutput":
            out_names.append(name)
            shape = tuple(alloc.tensor_shape)
            dtype = mybir.dt.np(alloc.dtype)
            out_avals.append(jax.core.ShapedArray(shape, dtype))
    n_params = len(in_names)
    n_outs = len(out_avals)
    all_in_names = list(in_names) + list(out_names)
    if partition_name:
        all_in_names.append(partition_name)

    def _body(*args):
        operands = list(args)
        if partition_name:
            operands.append(partition_id_tensor())
        return tuple(_bass_exec_p.bind(
            *operands, out_avals=tuple(out_avals), in_names=tuple(all_in_names),
            out_names=tuple(out_names), lowering_input_output_aliases=(),
            sim_require_finite=True, sim_require_nnan=True, nc=nc))

    devices = jax.devices()[:n_cores]
    mesh = Mesh(np.asarray(devices), ("core",))
    donate = tuple(range(n_params, n_params + n_outs))
    sharded = jax.jit(
        shard_map(_body, mesh=mesh,
                  in_specs=(PartitionSpec("core"),) * (n_params + n_outs),
                  out_specs=(PartitionSpec("core"),) * n_outs, check_rep=False),
        donate_argnums=donate, keep_unused=True)
    sharding = NamedSharding(mesh, PartitionSpec("core"))
    zero_shapes = [(n_cores * a.shape[0], *a.shape[1:]) for a in out_avals]
    zero_dtypes = [a.dtype for a in out_avals]

    def run(in_maps, pre=None):
        """in_maps: per-core dicts of np arrays. pre: dict name -> global jax
        Array (already sharded) taking precedence over in_maps."""
        import jax as _jax
        pre = pre or {}
        concat_in = []
        for name in in_names:
            if name in pre:
                concat_in.append(pre[name])
            else:
                concat_in.append(np.concatenate(
                    [np.asarray(m[name]) for m in in_maps], axis=0))
        zs = [_jax.device_put(np.zeros(s, d), sharding)
              for s, d in zip(zero_shapes, zero_dtypes)]
        outs = _jax.block_until_ready(sharded(*concat_in, *zs))
        return [{name: np.asarray(outs[i]).reshape(n_cores, *out_avals[i].shape)[c]
                 for i, name in enumerate(out_names)}
                for c in range(n_cores)]

    run.devices = devices
    run.sharding = sharding
    return run


def _get_programs():
    if "main" not in _CACHE:
        _CACHE["main"] = build_main_program()
        _CACHE["main_run"] = make_runner(_CACHE["main"])
    return _CACHE


# ----------------------------------------------------------------------------
# Entry point
# ----------------------------------------------------------------------------

def kernel(x, cluster_assignments, batch, Wk, bk, Wv, bv, Wo, bo, pool_query):
    import ml_dtypes
    bf16 = ml_dtypes.bfloat16

    x = np.ascontiguousarray(np.asarray(x, dtype=np.float32))
    ca = np.asarray(cluster_assignments).astype(np.int64)
    Wk = np.asarray(Wk, np.float32)
    bk = np.asarray(bk, np.float32)
    pq = np.asarray(pool_query, np.float32)[0]  # [H, HD]

    # folded score projection (tiny): scores = x @ As + c0
    As = (np.asarray(Wk, np.float64).reshape(HEADS, HD, HID)
          * np.asarray(pq, np.float64)[:, :, None]).sum(1)     # [H, HID]
    As = (As.T * SCALE).astype(np.float32)                     # [HID, H]
    c0 = ((np.asarray(bk, np.float64).reshape(HEADS, HD)
           * np.asarray(pq, np.float64)).sum(1) * SCALE).astype(np.float32)
    e = np.exp(x @ As + c0)                                    # [N, 8] f32

    progs = _get_programs()
    run = progs["main_run"]

    node_idx, segw, slot_cluster, counts = plan_sharding(ca)

    xpad = np.vstack([x, np.zeros((1, HID), np.float32)])
    epad = np.vstack([e, np.zeros((1, HEADS), np.float32)])
    nip = np.where(node_idx >= 0, node_idx, N).reshape(-1)     # [8*NL]

    # upload rows: [x(256) | 1 | e(8) | 0pad] in bf16
    x_big = np.empty((N_CORES * NL, COLS), bf16)
    x_big[:, 0:HID] = xpad[nip]
    x_big[:, HID] = 1.0
    x_big[:, HID + 1:HID + 9] = epad[nip]
    x_big[:, HID + 9] = 0.0
    # swizzle: rows of each ch-tile chunk so partition p's DMA is contiguous
    x_big = np.ascontiguousarray(
        x_big.reshape(N_CORES, NTILES // CH, CH, TILE, COLS)
        .transpose(0, 1, 3, 2, 4)
        .reshape(N_CORES * (NTILES // CH) * TILE, CH * COLS))

    win16 = np.repeat(np.arange(SLOTS, dtype=np.float32), HEADS)[None, :] \
        .repeat(TILE, 0).astype(bf16)                           # [128, 128]
    win16 = np.ascontiguousarray(np.tile(win16, (1, max(2, CH))))
    in_maps = []
    for core in range(N_CORES):
        segw_core = np.ascontiguousarray(
            segw[core].reshape(NTILES, TILE).T).astype(bf16)    # [128, NTILES]
        in_maps.append({"segw": segw_core, "win16": win16})

    results = run(in_maps, pre={"x": x_big})
    U_all = np.stack([results[i]["U"] for i in range(N_CORES)])
    z_all = np.stack([results[i]["z"] for i in range(N_CORES)])

    table = host_table_math(U_all, z_all, slot_cluster, counts, Wv, bv, Wo, bo)

    out = np.empty((N, HID), dtype=np.float32)
    np.take(table, ca, axis=0, out=out)
    return out


# revision 19
# speedup vs baseline: 10.4722x; 1.1317x over previous
"""Trainium2 Bass kernel for nn_AttentionPooling (segment-softmax attention pooling).

Math restructuring (vs the reference):
  scores[n,h] = (x @ Wk.T + bk) . pool_query  * scale  ==  x @ As + c0
      with As[j,h] = scale * sum_d Wk[h*HD+d, j] * pq[h,d]   (tiny [256,8])
  e = exp(scores)            (no max-subtraction needed; |scores| < ~6)
  z[c,h]   = sum_{n in c} e[n,h]
  U[c,h,:] = sum_{n in c} e[n,h] * x[n,:]
  T = U / z;  ssum[c, h*HD:(h+1)*HD] = T[c,h,:] @ Wv_h.T + bv_h   (softmax weights
      sum to 1 per segment, so the bias term is exact)
  pooled = ssum / max(cnt,1);  table = pooled @ Wo.T + bo;  out = table[seg]

Division of labor:
  Host (cheap O(N*8) / O(C) work): the tiny rank-8 score projection (one BLAS
      sgemm) + exp, cluster bin-packing, building the bf16 upload buffer, the
      [C, HID] table math, and the final out = table[seg] expansion.
  Device (the memory-bound segment reduction over all of x, SPMD on 8 cores):
      per 128-node tile, DVE builds the "weighted indicator"
      wind[n, slot*8+h] = (segw[n]==slot) * e[n,h] from the DMA'd e columns
      (batched per chunk as three 2x-mode tensor_tensor ops), and PE
      accumulates U/z with one bf16 matmul per tile:
      uz[slot*8+h, :] += wind.T @ [x | 1].  Clusters are host-bin-packed into
      8-per-core windows of <=16 clusters, each padded to exactly T_W tiles.

Everything the device touches is bf16 (x, e, masks): halves HBM traffic (the
bottleneck, ~368 GB/s/core) and enables the DVE 2x mode; accumulation stays
fp32 in PSUM.  The upload buffer is row-swizzled per chunk so each SBUF
partition's DMA is one contiguous read; chunk sizes taper at the sweep's ends
to shrink pipeline fill/drain, chunk DMAs alternate between the two HWDGE
rings (SP + ACT), and each window's U/z lands in HBM right after its last
accumulation, overlapped with the remaining sweep.
"""

import sys

if "/opt/trn_rl_repo" not in sys.path:
    sys.path.insert(0, "/opt/trn_rl_repo")

import numpy as np
from contextlib import ExitStack

import concourse.bass as bass
import concourse.tile as tile
from concourse import bacc, mybir

F32 = mybir.dt.float32
BF16 = mybir.dt.bfloat16

# Problem constants
N = 200000
HID = 256
HEADS = 8
HD = HID // HEADS
C = 1000
SCALE = HD ** -0.5

# Sharding constants
N_CORES = 8
TILE = 128
SLOTS = 16          # clusters per window
N_WIN = 8           # windows per core  (8*16*8 = 1024 cluster slots >= 1000)
T_W = 25            # tiles per window (padded; falls back to 26 if packing fails)
PAD_SLOT = 255.0

COLS = HID + 10     # per-row upload: 256 x | 1 ones | 8 e | 1 pad  (532 B)
UC = HID + 1        # U row: 256 U | 1 z


def chunk_plan(ntiles, big=20):
    """Tapered chunk sizes: small at both ends (pipeline fill/drain), big in
    the middle. Sums to ntiles."""
    head = [4, 6, 10]
    tail = [10, 6, 4]
    mid = ntiles - sum(head) - sum(tail)
    assert mid >= 0 and mid % big == 0, (ntiles, big)
    return head + [big] * (mid // big) + tail


# ----------------------------------------------------------------------------
# Device program
# ----------------------------------------------------------------------------

def build_main_program(n_win=N_WIN, t_w=T_W, repeat=1, hw_loop=0, stage="full",
                       big=20, x_bufs=4, wind_bufs=6, dma_alt=True,
                       slots=SLOTS, uout=True):
    """Single-sweep program over n_win*t_w 128-node tiles (see module doc).
    `repeat`/`hw_loop` re-run the whole sweep (for timing); `stage` truncates
    the per-tile pipeline ("dma" / "mask" / "full")."""
    ntiles = n_win * t_w
    srows = slots * HEADS
    chunks = chunk_plan(ntiles, big)
    nc = bacc.Bacc("TRN2", target_bir_lowering=False, debug=False,
                   enable_asserts=False, num_devices=N_CORES)

    # host-swizzled flat upload: per chunk, partition p's rows are contiguous
    x_d = nc.dram_tensor("x", [ntiles * TILE * COLS], BF16,
                         kind="ExternalInput").ap()
    segw_d = nc.dram_tensor("segw", [TILE, ntiles], BF16, kind="ExternalInput").ap()
    win16_d = nc.dram_tensor("win16", [TILE, max(c * srows for c in chunks)],
                             BF16, kind="ExternalInput").ap()
    U_d = nc.dram_tensor("U", [srows, n_win * UC], F32, kind="ExternalOutput").ap()

    with tile.TileContext(nc) as tc, ExitStack() as ctx:
        consts = ctx.enter_context(tc.tile_pool(name="consts", bufs=1))
        xs_pool = ctx.enter_context(tc.tile_pool(name="xcs", bufs=3))
        xl_pool = ctx.enter_context(tc.tile_pool(name="xcl", bufs=x_bufs))
        mask_pool = ctx.enter_context(tc.tile_pool(name="mask", bufs=wind_bufs))
        wind_pool = ctx.enter_context(tc.tile_pool(name="wind", bufs=wind_bufs))
        sg_pool = ctx.enter_context(tc.tile_pool(name="sg", bufs=3))
        u_pool = ctx.enter_context(tc.tile_pool(name="usb", bufs=3))
        uz_psum = ctx.enter_context(tc.tile_pool(name="uzps", bufs=3, space="PSUM"))

        win16_sb = consts.tile([TILE, max(c * srows for c in chunks)], BF16)
        nc.sync.dma_start(win16_sb[:], win16_d[:])
        segw_sb = consts.tile([TILE, ntiles], BF16)
        nc.sync.dma_start(segw_sb[:], segw_d[:])

        if stage != "full":
            zero_sb = consts.tile([srows, UC], F32)
            nc.vector.memset(zero_sb[:], 0.0)
            for w in range(n_win):
                nc.sync.dma_start(U_d[:, w * UC:(w + 1) * UC], zero_sb[:])

        def sweep():
            uz_cur = None
            t0 = 0
            for chb, ch in enumerate(chunks):
                pool = xl_pool if ch >= big else xs_pool
                xc = pool.tile([TILE, ch * COLS], BF16, tag=f"xc{ch}")
                dma_eng = nc.scalar if (dma_alt and chb % 2) else nc.sync
                src = x_d[t0 * TILE * COLS:(t0 + ch) * TILE * COLS] \
                    .rearrange("(p c) -> p c", p=TILE)
                dma_eng.dma_start(xc[:], src)
                if stage == "dma":
                    t0 += ch
                    continue
                # batched DVE: 3 ops per chunk, all TT-2x-eligible
                # (every operand's innermost dim is the contiguous h)
                sg8 = sg_pool.tile([TILE, ch * HEADS], BF16, tag=f"sg{ch}")
                sgv = sg8[:].rearrange("p (a h) -> p a h", a=ch)
                nc.vector.tensor_copy(
                    sgv, segw_sb[:, t0:t0 + ch].unsqueeze(-1)
                    .to_broadcast([TILE, ch, HEADS]))
                mask8 = mask_pool.tile([TILE, ch * srows], BF16, tag=f"m{ch}")
                m4 = mask8[:].rearrange("p (a s h) -> p a s h", a=ch, s=slots)
                win_v = win16_sb[:, 0:ch * srows].rearrange(
                    "p (a s h) -> p a s h", a=ch, s=slots)
                sg_b = sgv.unsqueeze(2).to_broadcast([TILE, ch, slots, HEADS])
                nc.vector.tensor_tensor(m4, win_v, sg_b,
                                        op=mybir.AluOpType.is_equal)
                wind8 = wind_pool.tile([TILE, ch * srows], BF16, tag=f"w{ch}")
                w4 = wind8[:].rearrange("p (a s h) -> p a s h", a=ch, s=slots)
                ev = xc[:].rearrange("p (a j) -> p a j", a=ch) \
                    [:, :, HID + 1:HID + 9].unsqueeze(2) \
                    .to_broadcast([TILE, ch, slots, HEADS])
                nc.vector.tensor_tensor(w4, m4, ev, op=mybir.AluOpType.mult)
                if stage == "mask":
                    t0 += ch
                    continue
                for a in range(ch):
                    t = t0 + a
                    w, ti = divmod(t, t_w)
                    base = a * COLS
                    if ti == 0:
                        uz_cur = uz_psum.tile([srows, UC], F32, tag="uz")
                    nc.tensor.matmul(uz_cur[:],
                                     lhsT=wind8[:, a * srows:(a + 1) * srows],
                                     rhs=xc[:, base:base + UC],
                                     start=(ti == 0), stop=(ti == t_w - 1))
                    if ti == t_w - 1:
                        usb = u_pool.tile([srows, UC], F32, tag="usb")
                        nc.scalar.copy(usb[:], uz_cur[:])
                        if uout:
                            ueng = nc.sync if (dma_alt and w % 2) else nc.scalar
                            ueng.dma_start(U_d[:, w * UC:(w + 1) * UC], usb[:])
                t0 += ch

        if hw_loop > 1:
            with tc.For_i(0, hw_loop, 1):
                sweep()
        else:
            for _rep in range(repeat):
                sweep()

    nc.compile()
    return nc


# Rebind the builder from source exec'd under a fixed pseudo-filename: the
# BIR embeds op debug info (filename/lineno), so building from the real file
# path would make the compiled program differ per directory and miss the
# neuronxcc compile cache.
import inspect as _inspect

_builder_src = (_inspect.getsource(chunk_plan) + "\n\n"
                + _inspect.getsource(build_main_program))
exec(compile(_builder_src, "<akp_builders>", "exec"), globals())


# ----------------------------------------------------------------------------
# Host-side planning
# ----------------------------------------------------------------------------

def plan_sharding(ca, t_w):
    """Bin-pack 1000 clusters into 64 (core, window) bins, <=16 clusters and
    <= t_w*128 nodes per bin. Returns per-core node index arrays + slot maps.
    Raises AssertionError if the packing does not fit."""
    nl = N_WIN * t_w * TILE
    counts = np.bincount(ca, minlength=C)
    order = np.argsort(counts, kind="stable")[::-1]
    nbins = N_CORES * N_WIN
    cap = t_w * TILE
    loads = np.zeros(nbins, dtype=np.int64)
    nslots = np.zeros(nbins, dtype=np.int64)
    bin_clusters = [[] for _ in range(nbins)]
    for c in order:
        # least-loaded bin with a free slot
        cand = np.where(nslots < SLOTS)[0]
        b = cand[np.argmin(loads[cand])]
        bin_clusters[b].append(int(c))
        loads[b] += counts[c]
        nslots[b] += 1
    assert loads.max() <= cap, f"bin overflow: {loads.max()} > {cap}"

    # node lists per cluster (sorted order)
    idx_sorted = np.argsort(ca, kind="stable")
    starts = np.zeros(C + 1, dtype=np.int64)
    np.cumsum(counts, out=starts[1:])

    node_idx = np.full((N_CORES, nl), -1, dtype=np.int64)
    segw = np.full((N_CORES, nl), PAD_SLOT, dtype=np.float32)
    slot_cluster = np.full((N_CORES, N_WIN, SLOTS), -1, dtype=np.int64)
    for b in range(nbins):
        core, w = divmod(b, N_WIN)
        pos = w * cap
        for s, c in enumerate(bin_clusters[b]):
            slot_cluster[core, w, s] = c
            m = counts[c]
            node_idx[core, pos:pos + m] = idx_sorted[starts[c]:starts[c] + m]
            segw[core, pos:pos + m] = s
            pos += m
    return node_idx, segw, slot_cluster, counts


def host_table_math(U_all, z_all, slot_cluster, counts, Wv, bv, Wo, bo):
    """[8,128,N_WIN*256] U + [8,128,N_WIN] z -> projected table [C, HID]."""
    # U[core][row=s*8+h, col=w*256+j] ; z[core][row, w]
    U5 = U_all.reshape(N_CORES, SLOTS, HEADS, N_WIN, HID)
    z4 = z_all.reshape(N_CORES, SLOTS, HEADS, N_WIN)
    Uc = np.zeros((C, HEADS, HID), dtype=np.float64)
    zc = np.zeros((C, HEADS), dtype=np.float64)
    sc = slot_cluster  # [core, w, s]
    valid = sc >= 0
    cores, ws, ss = np.nonzero(valid)
    cl = sc[cores, ws, ss]
    Uc[cl] = U5[cores, ss, :, ws, :]
    zc[cl] = z4[cores, ss, :, ws]
    zc_safe = np.where(zc > 0, zc, 1.0)
    T = Uc / zc_safe[:, :, None]                      # [C, H, HID]
    Wv_r = np.asarray(Wv, np.float64).reshape(HEADS, HD, HID)
    ssum = np.einsum("chj,hdj->chd", T, Wv_r)         # [C, H, HD]
    ssum += np.asarray(bv, np.float64).reshape(HEADS, HD)[None]
    ssum = ssum.reshape(C, HID)
    ssum[counts == 0] = 0.0
    pooled = ssum / np.maximum(counts, 1)[:, None]
    table = pooled @ np.asarray(Wo, np.float64).T + np.asarray(bo, np.float64)
    return table.astype(np.float32)


_CACHE = {}


def make_runner(nc, n_cores=N_CORES):
    """Persistent jitted runner for a compiled Bacc program (axon/PJRT path).

    Same mechanism as run_bass_kernel_spmd's axon redirect (bass2jax), but the
    jitted executable is built once and reused, so steady-state calls skip
    retracing/lowering."""
    import jax
    from jax.sharding import Mesh, PartitionSpec, NamedSharding
    from jax.experimental.shard_map import shard_map
    from concourse.bass2jax import (_bass_exec_p, install_neuronx_cc_hook,
                                    partition_id_tensor)

    install_neuronx_cc_hook()
    in_names, out_names, out_avals = [], [], []
    partition_name = nc.partition_id_tensor.name if nc.partition_id_tensor else None
    for alloc in nc.m.functions[0].allocations:
        if not isinstance(alloc, mybir.MemoryLocationSet):
            continue
        name = alloc.memorylocations[0].name
        if alloc.kind == "ExternalInput":
            if name != partition_name:
                in_names.append(name)
        elif alloc.kind == "ExternalOutput":
            out_names.append(name)
            shape = tuple(alloc.tensor_shape)
            dtype = mybir.dt.np(alloc.dtype)
            out_avals.append(jax.core.ShapedArray(shape, dtype))
    n_params = len(in_names)
    n_outs = len(out_avals)
    all_in_names = list(in_names) + list(out_names)
    if partition_name:
        all_in_names.append(partition_name)

    def _body(*args):
        operands = list(args)
        if partition_name:
            operands.append(partition_id_tensor())
        return tuple(_bass_exec_p.bind(
            *operands, out_avals=tuple(out_avals), in_names=tuple(all_in_names),
            out_names=tuple(out_names), lowering_input_output_aliases=(),
            sim_require_finite=True, sim_require_nnan=True, nc=nc))

    devices = jax.devices()[:n_cores]
    mesh = Mesh(np.asarray(devices), ("core",))
    donate = tuple(range(n_params, n_params + n_outs))
    sharded = jax.jit(
        shard_map(_body, mesh=mesh,
                  in_specs=(PartitionSpec("core"),) * (n_params + n_outs),
                  out_specs=(PartitionSpec("core"),) * n_outs, check_rep=False),
        donate_argnums=donate, keep_unused=True)
    sharding = NamedSharding(mesh, PartitionSpec("core"))
    zero_shapes = [(n_cores * a.shape[0], *a.shape[1:]) for a in out_avals]
    zero_dtypes = [a.dtype for a in out_avals]

    def run(in_maps, pre=None):
        """in_maps: per-core dicts of np arrays. pre: dict name -> global jax
        Array (already sharded) taking precedence over in_maps."""
        import jax as _jax
        pre = pre or {}
        concat_in = []
        for name in in_names:
            if name in pre:
                concat_in.append(pre[name])
            else:
                concat_in.append(np.concatenate(
                    [np.asarray(m[name]) for m in in_maps], axis=0))
        zs = [_jax.device_put(np.zeros(s, d), sharding)
              for s, d in zip(zero_shapes, zero_dtypes)]
        outs = _jax.block_until_ready(sharded(*concat_in, *zs))
        return [{name: np.asarray(outs[i]).reshape(n_cores, *out_avals[i].shape)[c]
                 for i, name in enumerate(out_names)}
                for c in range(n_cores)]

    run.devices = devices
    run.sharding = sharding
    return run


def _get_program(t_w):
    key = f"main{t_w}"
    if key not in _CACHE:
        _CACHE[key] = build_main_program(t_w=t_w)
        _CACHE[key + "_run"] = make_runner(_CACHE[key])
    return _CACHE[key], _CACHE[key + "_run"]


# ----------------------------------------------------------------------------
# Entry point
# ----------------------------------------------------------------------------

def kernel(x, cluster_assignments, batch, Wk, bk, Wv, bv, Wo, bo, pool_query):
    import ml_dtypes
    bf16 = ml_dtypes.bfloat16

    x = np.ascontiguousarray(np.asarray(x, dtype=np.float32))
    ca = np.asarray(cluster_assignments).astype(np.int64)
    Wk = np.asarray(Wk, np.float32)
    bk = np.asarray(bk, np.float32)
    pq = np.asarray(pool_query, np.float32)[0]  # [H, HD]

    # folded score projection (tiny): scores = x @ As + c0
    As = (np.asarray(Wk, np.float64).reshape(HEADS, HD, HID)
          * np.asarray(pq, np.float64)[:, :, None]).sum(1)     # [H, HID]
    As = (As.T * SCALE).astype(np.float32)                     # [HID, H]
    c0 = ((np.asarray(bk, np.float64).reshape(HEADS, HD)
           * np.asarray(pq, np.float64)).sum(1) * SCALE).astype(np.float32)
    e = np.exp(x @ As + c0)                                    # [N, 8] f32

    # pack clusters; fall back to a roomier layout if the tight one overflows
    for t_w in (T_W, T_W + 1, T_W + 2):
        try:
            node_idx, segw, slot_cluster, counts = plan_sharding(ca, t_w)
            break
        except AssertionError:
            continue
    else:
        raise RuntimeError("cluster packing failed")
    prog, run = _get_program(t_w)
    ntiles = N_WIN * t_w
    nl = ntiles * TILE
    chunks = chunk_plan(ntiles)

    xpad = np.vstack([x, np.zeros((1, HID), np.float32)])
    epad = np.vstack([e, np.zeros((1, HEADS), np.float32)])
    nip = np.where(node_idx >= 0, node_idx, N).reshape(-1)     # [8*nl]

    # upload rows: [x(256) | 1 | e(8) | 0pad] in bf16
    x_big = np.empty((N_CORES * nl, COLS), bf16)
    x_big[:, 0:HID] = xpad[nip]
    x_big[:, HID] = 1.0
    x_big[:, HID + 1:HID + 9] = epad[nip]
    x_big[:, HID + 9] = 0.0
    # per-chunk swizzle: rows so each partition's chunk DMA is contiguous
    x_sw = np.empty((N_CORES, nl * COLS), bf16)
    xb = x_big.reshape(N_CORES, ntiles, TILE, COLS)
    t0 = 0
    for ch in chunks:
        seg = xb[:, t0:t0 + ch].transpose(0, 2, 1, 3)          # [8, 128, ch, COLS]
        x_sw[:, t0 * TILE * COLS:(t0 + ch) * TILE * COLS] = \
            seg.reshape(N_CORES, -1)
        t0 += ch
    x_sw = x_sw.reshape(-1)

    wmax = max(c * SLOTS * HEADS for c in chunks)
    win16 = np.repeat(np.arange(SLOTS, dtype=np.float32), HEADS)[None, :] \
        .repeat(TILE, 0).astype(bf16)                           # [128, 128]
    win16 = np.ascontiguousarray(np.tile(win16, (1, wmax // (SLOTS * HEADS))))
    in_maps = []
    for core in range(N_CORES):
        segw_core = np.ascontiguousarray(
            segw[core].reshape(ntiles, TILE).T).astype(bf16)    # [128, ntiles]
        in_maps.append({"segw": segw_core, "win16": win16})

    results = run(in_maps, pre={"x": x_sw})
    UZ = np.stack([results[i]["U"] for i in range(N_CORES)])    # [8,128,N_WIN*257]
    UZ = UZ.reshape(N_CORES, SLOTS * HEADS, N_WIN, UC)
    U_all = np.ascontiguousarray(UZ[:, :, :, 0:HID]).reshape(
        N_CORES, SLOTS * HEADS, N_WIN * HID)
    z_all = np.ascontiguousarray(UZ[:, :, :, HID])

    table = host_table_math(U_all, z_all, slot_cluster, counts, Wv, bv, Wo, bo)

    out = np.empty((N, HID), dtype=np.float32)
    np.take(table, ca, axis=0, out=out)
    return out


# revision 27
# speedup vs baseline: 12.4545x; 1.1893x over previous
"""Trainium2 Bass kernel for nn_AttentionPooling (segment-softmax attention pooling).

Math restructuring (vs the reference):
  scores[n,h] = (x @ Wk.T + bk) . pool_query  * scale  ==  x @ As + c0
      with As[j,h] = scale * sum_d Wk[h*HD+d, j] * pq[h,d]   (tiny [256,8])
  e = exp(scores)            (no max-subtraction needed; |scores| < ~6)
  z[c,h]   = sum_{n in c} e[n,h]
  U[c,h,:] = sum_{n in c} e[n,h] * x[n,:]
  T = U / z;  ssum[c, h*HD:(h+1)*HD] = T[c,h,:] @ Wv_h.T + bv_h   (softmax weights
      sum to 1 per segment, so the bias term is exact)
  pooled = ssum / max(cnt,1);  table = pooled @ Wo.T + bo;  out = table[seg]

Division of labor:
  Host (cheap O(N*8) / O(C) work): the tiny rank-8 score projection (one BLAS
      sgemm) + exp, cluster bin-packing, building the bf16 upload buffer, the
      [C, HID] table math, and the final out = table[seg] expansion.
  Device (the memory-bound segment reduction over all of x, SPMD on 8 cores):
      per 128-node tile, DVE builds the "weighted indicator"
      wind[n, slot*8+h] = (segw[n]==slot) * e[n,h] from the DMA'd e columns
      (batched per chunk as three 2x-mode tensor_tensor ops), and PE
      accumulates U/z with one bf16 matmul per tile:
      uz[slot*8+h, :] += wind.T @ [x | 1].  Clusters are host-bin-packed into
      8-per-core windows of <=16 clusters, each padded to exactly T_W tiles.

Everything the device touches is bf16 (x, e, masks): halves HBM traffic (the
bottleneck, ~368 GB/s/core) and enables the DVE 2x mode; accumulation stays
fp32 in PSUM.  The upload buffer is row-swizzled per chunk so each SBUF
partition's DMA is one contiguous read; chunk sizes taper at the sweep's ends
to shrink pipeline fill/drain, chunk DMAs alternate between the two HWDGE
rings (SP + ACT), and each window's U/z lands in HBM right after its last
accumulation, overlapped with the remaining sweep.
"""

import sys

if "/opt/trn_rl_repo" not in sys.path:
    sys.path.insert(0, "/opt/trn_rl_repo")

import numpy as np
from contextlib import ExitStack

import concourse.bass as bass
import concourse.tile as tile
from concourse import bacc, mybir

F32 = mybir.dt.float32
BF16 = mybir.dt.bfloat16

# Problem constants
N = 200000
HID = 256
HEADS = 8
HD = HID // HEADS
C = 1000
SCALE = HD ** -0.5

# Sharding constants
N_CORES = 8
TILE = 128
SLOTS = 16          # clusters per window
N_WIN = 8           # windows per core  (8*16*8 = 1024 cluster slots >= 1000)
T_W = 25            # tiles per window (padded; falls back to 26 if packing fails)
PAD_SLOT = 255.0

COLS = HID + 8      # per-row upload: 256 x | 8 e  (528 B)
UC = HID            # U row (z is recomputed on host from the same bf16 e)


def chunk_plan(ntiles, big=20):
    """Tapered chunk sizes: small at both ends (pipeline fill/drain), big in
    the middle. Sums to ntiles."""
    head = [4, 6, 10]
    tail = [10, 6, 4]
    mid = ntiles - sum(head) - sum(tail)
    assert mid >= 0 and mid % big == 0, (ntiles, big)
    return head + [big] * (mid // big) + tail


# ----------------------------------------------------------------------------
# Device program
# ----------------------------------------------------------------------------

def build_main_program(n_win=N_WIN, t_w=T_W, repeat=1, hw_loop=0, stage="full",
                       big=20, x_bufs=4, wind_bufs=6, dma_alt=True,
                       slots=SLOTS, uearly=True):
    """Single-sweep program over n_win*t_w 128-node tiles (see module doc).
    `repeat`/`hw_loop` re-run the whole sweep (for timing); `stage` truncates
    the per-tile pipeline ("dma" / "mask" / "full")."""
    ntiles = n_win * t_w
    srows = slots * HEADS
    chunks = chunk_plan(ntiles, big)
    nc = bacc.Bacc("TRN2", target_bir_lowering=False, debug=False,
                   enable_asserts=False, num_devices=N_CORES)

    # host-swizzled flat upload: per chunk, partition p's rows are contiguous
    x_d = nc.dram_tensor("x", [ntiles * TILE * COLS], BF16,
                         kind="ExternalInput").ap()
    segw_d = nc.dram_tensor("segw", [TILE, ntiles], BF16, kind="ExternalInput").ap()
    win16_d = nc.dram_tensor("win16", [TILE, max(c * srows for c in chunks)],
                             BF16, kind="ExternalInput").ap()
    U_d = nc.dram_tensor("U", [srows, n_win * UC], F32, kind="ExternalOutput").ap()

    with tile.TileContext(nc) as tc, ExitStack() as ctx:
        consts = ctx.enter_context(tc.tile_pool(name="consts", bufs=1))
        big_cols = max(chunks)
        x_pool = ctx.enter_context(tc.tile_pool(name="xc", bufs=x_bufs))
        mask_pool = ctx.enter_context(tc.tile_pool(name="mask", bufs=wind_bufs))
        wind_pool = ctx.enter_context(tc.tile_pool(name="wind", bufs=wind_bufs))
        sg_pool = ctx.enter_context(tc.tile_pool(name="sg", bufs=3))
        u_pool = ctx.enter_context(tc.tile_pool(name="usb", bufs=3))
        uz_psum = ctx.enter_context(tc.tile_pool(name="uzps", bufs=3, space="PSUM"))

        win16_sb = consts.tile([TILE, max(c * srows for c in chunks)], BF16)
        nc.sync.dma_start(win16_sb[:], win16_d[:])
        segw_sb = consts.tile([TILE, ntiles], BF16)
        nc.sync.dma_start(segw_sb[:], segw_d[:])

        if stage != "full":
            zero_sb = consts.tile([srows, UC], F32)
            nc.vector.memset(zero_sb[:], 0.0)
            for w in range(n_win):
                nc.sync.dma_start(U_d[:, w * UC:(w + 1) * UC], zero_sb[:])
        U_sb = consts.tile([srows, n_win * UC], F32)

        def sweep():
            uz_cur = None
            t0 = 0
            for chb, ch in enumerate(chunks):
                xcf = x_pool.tile([TILE, big_cols * COLS], BF16, tag="xc")
                xc = xcf[:, 0:ch * COLS]
                # tail chunks go on the SP ring only, freeing the ACT ring
                # for the early U writeback
                in_tail = t0 + ch > (n_win - 1) * t_w
                dma_eng = nc.scalar if (dma_alt and chb % 2 and not in_tail) \
                    else nc.sync
                src = x_d[t0 * TILE * COLS:(t0 + ch) * TILE * COLS] \
                    .rearrange("(p c) -> p c", p=TILE)
                dma_eng.dma_start(xc, src)
                if stage == "dma":
                    t0 += ch
                    continue
                # batched DVE: 3 ops per chunk, all TT-2x-eligible
                # (every operand's innermost dim is the contiguous h)
                sg8 = sg_pool.tile([TILE, big_cols * HEADS], BF16, tag="sg")
                sgv = sg8[:, 0:ch * HEADS].rearrange("p (a h) -> p a h", a=ch)
                nc.vector.tensor_copy(
                    sgv, segw_sb[:, t0:t0 + ch].unsqueeze(-1)
                    .to_broadcast([TILE, ch, HEADS]))
                mask8 = mask_pool.tile([TILE, big_cols * srows], BF16, tag="m")
                m4 = mask8[:, 0:ch * srows].rearrange(
                    "p (a s h) -> p a s h", a=ch, s=slots)
                win_v = win16_sb[:, 0:ch * srows].rearrange(
                    "p (a s h) -> p a s h", a=ch, s=slots)
                sg_b = sgv.unsqueeze(2).to_broadcast([TILE, ch, slots, HEADS])
                nc.vector.tensor_tensor(m4, win_v, sg_b,
                                        op=mybir.AluOpType.is_equal)
                wind8 = wind_pool.tile([TILE, big_cols * srows], BF16, tag="w")
                w4 = wind8[:, 0:ch * srows].rearrange(
                    "p (a s h) -> p a s h", a=ch, s=slots)
                ev = xc.rearrange("p (a j) -> p a j", a=ch) \
                    [:, :, HID:HID + 8].unsqueeze(2) \
                    .to_broadcast([TILE, ch, slots, HEADS])
                nc.vector.tensor_tensor(w4, m4, ev, op=mybir.AluOpType.mult)
                if stage == "mask":
                    t0 += ch
                    continue
                for a in range(ch):
                    t = t0 + a
                    w, ti = divmod(t, t_w)
                    base = a * COLS
                    if ti == 0:
                        uz_cur = uz_psum.tile([srows, UC], F32, tag="uz")
                    nc.tensor.matmul(uz_cur[:],
                                     lhsT=wind8[:, a * srows:(a + 1) * srows],
                                     rhs=xcf[:, base:base + UC],
                                     start=(ti == 0), stop=(ti == t_w - 1))
                    if ti == t_w - 1:
                        nc.scalar.copy(U_sb[:, w * UC:(w + 1) * UC],
                                       uz_cur[:])
                        if stage == "full" and uearly and w == n_win - 2:
                            nc.scalar.dma_start(
                                U_d[:, 0:(n_win - 1) * UC],
                                U_sb[:, 0:(n_win - 1) * UC])
                t0 += ch
            if stage == "full":
                if uearly:
                    nc.scalar.dma_start(U_d[:, (n_win - 1) * UC:],
                                        U_sb[:, (n_win - 1) * UC:])
                else:
                    nc.sync.dma_start(U_d[:], U_sb[:])

        if hw_loop > 1:
            with tc.For_i(0, hw_loop, 1):
                sweep()
        else:
            for _rep in range(repeat):
                sweep()

    nc.compile()
    return nc


# Rebind the builder from source exec'd under a fixed pseudo-filename: the
# BIR embeds op debug info (filename/lineno), so building from the real file
# path would make the compiled program differ per directory and miss the
# neuronxcc compile cache.
import inspect as _inspect

_builder_src = (_inspect.getsource(chunk_plan) + "\n\n"
                + _inspect.getsource(build_main_program))
exec(compile(_builder_src, "<akp_builders>", "exec"), globals())


# ----------------------------------------------------------------------------
# Host-side planning
# ----------------------------------------------------------------------------

def plan_sharding(ca, t_w):
    """Bin-pack 1000 clusters into 64 (core, window) bins, <=16 clusters and
    <= t_w*128 nodes per bin. Returns per-core node index arrays + slot maps.
    Raises AssertionError if the packing does not fit."""
    nl = N_WIN * t_w * TILE
    counts = np.bincount(ca, minlength=C)
    order = np.argsort(counts, kind="stable")[::-1]
    nbins = N_CORES * N_WIN
    cap = t_w * TILE
    loads = np.zeros(nbins, dtype=np.int64)
    nslots = np.zeros(nbins, dtype=np.int64)
    bin_clusters = [[] for _ in range(nbins)]
    for c in order:
        # least-loaded bin with a free slot
        cand = np.where(nslots < SLOTS)[0]
        b = cand[np.argmin(loads[cand])]
        bin_clusters[b].append(int(c))
        loads[b] += counts[c]
        nslots[b] += 1
    assert loads.max() <= cap, f"bin overflow: {loads.max()} > {cap}"

    # node lists per cluster (sorted order)
    idx_sorted = np.argsort(ca, kind="stable")
    starts = np.zeros(C + 1, dtype=np.int64)
    np.cumsum(counts, out=starts[1:])

    node_idx = np.full((N_CORES, nl), -1, dtype=np.int64)
    segw = np.full((N_CORES, nl), PAD_SLOT, dtype=np.float32)
    slot_cluster = np.full((N_CORES, N_WIN, SLOTS), -1, dtype=np.int64)
    for b in range(nbins):
        core, w = divmod(b, N_WIN)
        pos = w * cap
        for s, c in enumerate(bin_clusters[b]):
            slot_cluster[core, w, s] = c
            m = counts[c]
            node_idx[core, pos:pos + m] = idx_sorted[starts[c]:starts[c] + m]
            segw[core, pos:pos + m] = s
            pos += m
    return node_idx, segw, slot_cluster, counts


def host_table_math(U_all, zc, slot_cluster, counts, Wv, bv, Wo, bo):
    """[8,128,N_WIN*256] U + host z [C,H] -> projected table [C, HID]."""
    # U[core][row=s*8+h, col=w*256+j]
    U5 = U_all.reshape(N_CORES, SLOTS, HEADS, N_WIN, HID)
    Uc = np.zeros((C, HEADS, HID), dtype=np.float64)
    sc = slot_cluster  # [core, w, s]
    valid = sc >= 0
    cores, ws, ss = np.nonzero(valid)
    cl = sc[cores, ws, ss]
    Uc[cl] = U5[cores, ss, :, ws, :]
    zc = np.asarray(zc, np.float64)
    zc_safe = np.where(zc > 0, zc, 1.0)
    T = Uc / zc_safe[:, :, None]                      # [C, H, HID]
    Wv_r = np.asarray(Wv, np.float64).reshape(HEADS, HD, HID)
    ssum = np.einsum("chj,hdj->chd", T, Wv_r)         # [C, H, HD]
    ssum += np.asarray(bv, np.float64).reshape(HEADS, HD)[None]
    ssum = ssum.reshape(C, HID)
    ssum[counts == 0] = 0.0
    pooled = ssum / np.maximum(counts, 1)[:, None]
    table = pooled @ np.asarray(Wo, np.float64).T + np.asarray(bo, np.float64)
    return table.astype(np.float32)


_CACHE = {}


def make_runner(nc, n_cores=N_CORES):
    """Persistent jitted runner for a compiled Bacc program (axon/PJRT path).

    Same mechanism as run_bass_kernel_spmd's axon redirect (bass2jax), but the
    jitted executable is built once and reused, so steady-state calls skip
    retracing/lowering."""
    import jax
    from jax.sharding import Mesh, PartitionSpec, NamedSharding
    from jax.experimental.shard_map import shard_map
    from concourse.bass2jax import (_bass_exec_p, install_neuronx_cc_hook,
                                    partition_id_tensor)

    install_neuronx_cc_hook()
    in_names, out_names, out_avals = [], [], []
    partition_name = nc.partition_id_tensor.name if nc.partition_id_tensor else None
    for alloc in nc.m.functions[0].allocations:
        if not isinstance(alloc, mybir.MemoryLocationSet):
            continue
        name = alloc.memorylocations[0].name
        if alloc.kind == "ExternalInput":
            if name != partition_name:
                in_names.append(name)
        elif alloc.kind == "ExternalOutput":
            out_names.append(name)
            shape = tuple(alloc.tensor_shape)
            dtype = mybir.dt.np(alloc.dtype)
            out_avals.append(jax.core.ShapedArray(shape, dtype))
    n_params = len(in_names)
    n_outs = len(out_avals)
    all_in_names = list(in_names) + list(out_names)
    if partition_name:
        all_in_names.append(partition_name)

    def _body(*args):
        operands = list(args)
        if partition_name:
            operands.append(partition_id_tensor())
        return tuple(_bass_exec_p.bind(
            *operands, out_avals=tuple(out_avals), in_names=tuple(all_in_names),
            out_names=tuple(out_names), lowering_input_output_aliases=(),
            sim_require_finite=True, sim_require_nnan=True, nc=nc))

    devices = jax.devices()[:n_cores]
    mesh = Mesh(np.asarray(devices), ("core",))
    donate = tuple(range(n_params, n_params + n_outs))
    sharded = jax.jit(
        shard_map(_body, mesh=mesh,
                  in_specs=(PartitionSpec("core"),) * (n_params + n_outs),
                  out_specs=(PartitionSpec("core"),) * n_outs, check_rep=False),
        donate_argnums=donate, keep_unused=True)
    sharding = NamedSharding(mesh, PartitionSpec("core"))
    zero_shapes = [(n_cores * a.shape[0], *a.shape[1:]) for a in out_avals]
    zero_dtypes = [a.dtype for a in out_avals]

    def run(in_maps, pre=None):
        """in_maps: per-core dicts of np arrays. pre: dict name -> global jax
        Array (already sharded) taking precedence over in_maps."""
        import jax as _jax
        pre = pre or {}
        concat_in = []
        for name in in_names:
            if name in pre:
                concat_in.append(pre[name])
            else:
                concat_in.append(np.concatenate(
                    [np.asarray(m[name]) for m in in_maps], axis=0))
        zs = [_jax.device_put(np.zeros(s, d), sharding)
              for s, d in zip(zero_shapes, zero_dtypes)]
        outs = _jax.block_until_ready(sharded(*concat_in, *zs))
        return [{name: np.asarray(outs[i]).reshape(n_cores, *out_avals[i].shape)[c]
                 for i, name in enumerate(out_names)}
                for c in range(n_cores)]

    run.devices = devices
    run.sharding = sharding
    return run


def _get_program(t_w):
    key = f"main{t_w}"
    if key not in _CACHE:
        _CACHE[key] = build_main_program(t_w=t_w)
        _CACHE[key + "_run"] = make_runner(_CACHE[key])
    return _CACHE[key], _CACHE[key + "_run"]


# ----------------------------------------------------------------------------
# Entry point
# ----------------------------------------------------------------------------

def kernel(x, cluster_assignments, batch, Wk, bk, Wv, bv, Wo, bo, pool_query):
    import ml_dtypes
    bf16 = ml_dtypes.bfloat16

    x = np.ascontiguousarray(np.asarray(x, dtype=np.float32))
    ca = np.asarray(cluster_assignments).astype(np.int64)
    Wk = np.asarray(Wk, np.float32)
    bk = np.asarray(bk, np.float32)
    pq = np.asarray(pool_query, np.float32)[0]  # [H, HD]

    # folded score projection (tiny): scores = x @ As + c0
    As = (np.asarray(Wk, np.float64).reshape(HEADS, HD, HID)
          * np.asarray(pq, np.float64)[:, :, None]).sum(1)     # [H, HID]
    As = (As.T * SCALE).astype(np.float32)                     # [HID, H]
    c0 = ((np.asarray(bk, np.float64).reshape(HEADS, HD)
           * np.asarray(pq, np.float64)).sum(1) * SCALE).astype(np.float32)
    e = np.exp(x @ As + c0)                                    # [N, 8] f32

    # pack clusters; fall back to a roomier layout if the tight one overflows
    for t_w in (T_W, T_W + 1, T_W + 2):
        try:
            node_idx, segw, slot_cluster, counts = plan_sharding(ca, t_w)
            break
        except AssertionError:
            continue
    else:
        raise RuntimeError("cluster packing failed")
    prog, run = _get_program(t_w)
    ntiles = N_WIN * t_w
    nl = ntiles * TILE
    chunks = chunk_plan(ntiles)

    xpad = np.vstack([x, np.zeros((1, HID), np.float32)])
    epad = np.vstack([e, np.zeros((1, HEADS), np.float32)])
    nip = np.where(node_idx >= 0, node_idx, N).reshape(-1)     # [8*nl]

    # upload rows: [x(256) | e(8)] in bf16
    x_big = np.empty((N_CORES * nl, COLS), bf16)
    x_big[:, 0:HID] = xpad[nip]
    x_big[:, HID:HID + 8] = epad[nip]
    # per-chunk swizzle: rows so each partition's chunk DMA is contiguous
    x_sw = np.empty((N_CORES, nl * COLS), bf16)
    xb = x_big.reshape(N_CORES, ntiles, TILE, COLS)
    t0 = 0
    for ch in chunks:
        seg = xb[:, t0:t0 + ch].transpose(0, 2, 1, 3)          # [8, 128, ch, COLS]
        x_sw[:, t0 * TILE * COLS:(t0 + ch) * TILE * COLS] = \
            seg.reshape(N_CORES, -1)
        t0 += ch
    x_sw = x_sw.reshape(-1)

    wmax = max(c * SLOTS * HEADS for c in chunks)
    win16 = np.repeat(np.arange(SLOTS, dtype=np.float32), HEADS)[None, :] \
        .repeat(TILE, 0).astype(bf16)                           # [128, 128]
    win16 = np.ascontiguousarray(np.tile(win16, (1, wmax // (SLOTS * HEADS))))
    in_maps = []
    for core in range(N_CORES):
        segw_core = np.ascontiguousarray(
            segw[core].reshape(ntiles, TILE).T).astype(bf16)    # [128, ntiles]
        in_maps.append({"segw": segw_core, "win16": win16})

    results = run(in_maps, pre={"x": x_sw})
    U_all = np.stack([results[i]["U"] for i in range(N_CORES)])  # [8,128,N_WIN*256]

    # z from the same bf16 e values the device used (consistent normalization)
    e_b64 = np.asarray(x_big[:, HID:HID + 8], np.float64)
    zc = np.zeros((C, HEADS))
    flat_idx = node_idx.reshape(-1)
    sel = flat_idx >= 0
    cl_of_row = ca[flat_idx[sel]]
    eb = e_b64[sel]
    for h in range(HEADS):
        zc[:, h] = np.bincount(cl_of_row, weights=eb[:, h], minlength=C)

    table = host_table_math(U_all, zc, slot_cluster, counts, Wv, bv, Wo, bo)

    out = np.empty((N, HID), dtype=np.float32)
    np.take(table, ca, axis=0, out=out)
    return out
